# revision 22
# baseline (speedup 1.0000x reference)
"""Trainium2 Bass kernel for DownSamplingSpatial2Channel (space-to-depth + projection).

Computes, for a dense 96^3 voxel grid with 64 channels:
    out[d] = sum_s in_data[r(d, s)] @ W_s
where d indexes the 48^3 coarse grid, s the 8 sub-voxels of a 2x2x2 block,
r(d, s) the fine-grid row, and W_s = w_out[64*s : 64*s+64, :].

Sharding: data-parallel over fine-grid i-planes. Core d owns fine planes
[12d, 12d+12) and coarse planes [6d, 6d+6).

Host prep (not on the device critical path): in_data is rounded to bf16 and
laid out channel-major with the k-subvoxel pair folded onto the partition
axis, blocked so each (coarse plane ci, j-chunk c0) is one contiguous
[128, 1536] DRAM block:
  x[d][ci*6 + c0][64*lk + c, li*768 + jj*48 + kc]
      = in_data[(12d + 2*ci + li, 16*c0 + jj, 2*kc + lk), c].
The [512,64] projection is regrouped as 4 stationaries W'_{li,lj} [128,64]
whose contraction dim spans (lk, c) — each matmul contracts a fully dense
K=128 (two sub-voxels at once) and the space-to-depth scatter is entirely
strided access patterns on the moving operand.

Device pipeline per core (36 chunks, uniform):
  1. One plain 384 KB DMA load per chunk ([128, 1536] bf16, alternating the
     two HWDGE rings) — contiguous, full HBM bandwidth, no XBAR. Chunk
     granularity keeps the PE no more than ~1 chunk behind the load stream,
     so the post-last-byte tail is one chunk's matmul + cast + store.
  2. Per chunk: 4 accumulating bf16 matmuls (K=128, N=384) into PSUM
     Y.T [64 out-ch, 384 voxels], then a DVE fp32->bf16 cast into SBUF.
  3. One [64, 2304] store per coarse plane on the SWDGE ring; the last
     coarse plane stores per chunk on the (by then idle) HWDGE rings.
     Output stays transposed ([64, Nd] bf16); the host transposes back and
     upcasts when unsharding.
"""

import numpy as np

D = 96            # fine grid edge
DS = 48           # coarse grid edge
C = 64            # channels
N_CORES = 8
PLANES_PER_CORE = D // N_CORES          # 12 fine i-planes
CI_PER_CORE = DS // N_CORES             # 6 coarse i-planes
ND = CI_PER_CORE * DS * DS              # 13824 coarse rows per core
CJ_BLK = 8                              # coarse j-lines per matmul chunk
NFREE = CJ_BLK * DS                     # 384 moving free dim
NCHUNKS_CJ = DS // CJ_BLK               # 6 chunks per coarse plane
CHUNK_COLS = 2 * 2 * CJ_BLK * (D // 2)  # 1536 packed cols per chunk
N_CHUNKS = CI_PER_CORE * NCHUNKS_CJ     # 36 chunks per core

_CACHE = {}


def build_nc(n_ci=CI_PER_CORE):
    from contextlib import ExitStack

    import concourse.bass as bass  # noqa: F401
    import concourse.mybir as mybir
    import concourse.tile as tile
    from concourse import bacc

    dt = mybir.dt
    f32, bf16 = dt.float32, dt.bfloat16
    n_chunks = n_ci * NCHUNKS_CJ
    nd = n_ci * DS * DS

    nc = bacc.Bacc(
        "TRN2",
        target_bir_lowering=False,
        debug=False,
        num_devices=N_CORES,
    )
    x = nc.dram_tensor("x", [n_chunks, 128, CHUNK_COLS], bf16, kind="ExternalInput").ap()
    # w slots t = 2*li + lj: stationary W'_t with K spanning (lk, c)
    w = nc.dram_tensor("w", [128, 4, C], bf16, kind="ExternalInput").ap()
    y = nc.dram_tensor("y", [C, nd], bf16, kind="ExternalOutput").ap()

    with tile.TileContext(nc) as tc, ExitStack() as ctx:
        const = ctx.enter_context(tc.tile_pool(name="const", bufs=1))
        xtpool = ctx.enter_context(tc.tile_pool(name="xt", bufs=n_chunks))
        ypool = ctx.enter_context(tc.tile_pool(name="ysb", bufs=2))
        apsum = ctx.enter_context(tc.tile_pool(name="acc", bufs=4, space="PSUM"))

        wt = const.tile([128, 4, C], bf16, tag="wt")
        nc.gpsimd.dma_start(out=wt[:], in_=w)

        # PE warm-up: dummy matmuls on a zeroed scratch tile keep the PE
        # busy through the HAM SHORT window during the load ramp, so the
        # first real matmuls run at 2.4 GHz instead of the cold 1.2 GHz.
        # The scratch PSUM result is never read.
        warm = const.tile([128, NFREE], bf16, tag="warm")
        nc.vector.memset(warm[:], 0)
        wacc = apsum.tile([C, NFREE], f32, tag="wacc")
        for _ in range(9):
            nc.tensor.matmul(
                wacc[:], warm[:, 0:C], warm[:], start=True, stop=True
            )

        xt_tiles = {}
        for k in range(n_chunks):
            xt = xtpool.tile([128, CHUNK_COLS], bf16, tag="xt")
            xt_tiles[k] = xt[:]
            eng = nc.sync if k % 2 == 0 else nc.scalar
            if k < n_chunks - 2:
                eng.dma_start(out=xt[:], in_=x[k])
            else:
                # Last two chunks: four quarter-loads each, so the DMA
                # completion receipt and most of the chunk's matmuls overlap
                # earlier quarters instead of sitting on the end-of-kernel
                # dependency chain.
                xk = x[k].rearrange("p (li j kc) -> p li j kc", li=2, j=2 * CJ_BLK)
                xtv = xt[:].rearrange("p (li j kc) -> p li j kc", li=2, j=2 * CJ_BLK)
                for s in range(4):
                    eng.dma_start(
                        out=xtv[:, :, 4 * s : 4 * (s + 1), :],
                        in_=xk[:, :, 4 * s : 4 * (s + 1), :],
                    )

        for ci in range(n_ci):
            last = ci == n_ci - 1
            ysb = ypool.tile([C, NCHUNKS_CJ * NFREE], bf16, tag="ysb")
            for c0 in range(NCHUNKS_CJ):
                k = ci * NCHUNKS_CJ + c0
                xt4 = xt_tiles[k].rearrange(
                    "p (li j kc) -> p li j kc", li=2, j=2 * CJ_BLK
                )
                ysl = ysb[:, NFREE * c0 : NFREE * (c0 + 1)]
                ybase = ci * DS * DS + NFREE * c0
                if k < n_chunks - 2:
                    acc = apsum.tile([C, NFREE], f32, tag="acc")
                    for li in range(2):
                        for lj in range(2):
                            t = 2 * li + lj
                            nc.tensor.matmul(
                                acc[:],
                                wt[:, t, :],
                                xt4[:, li, lj : 2 * CJ_BLK : 2, :],
                                start=(t == 0),
                                stop=(t == 3),
                            )
                    nc.vector.tensor_copy(out=ysl, in_=acc[:])
                else:
                    # Quarter-granular compute matching the quarter-loads.
                    nsub = NFREE // 4
                    for s in range(4):
                        acc = apsum.tile([C, nsub], f32, tag="acc")
                        for li in range(2):
                            for lj in range(2):
                                t = 2 * li + lj
                                nc.tensor.matmul(
                                    acc[:],
                                    wt[:, t, :],
                                    xt4[:, li, 4 * s + lj : 4 * s + 4 : 2, :],
                                    start=(t == 0),
                                    stop=(t == 3),
                                )
                        nc.vector.tensor_copy(
                            out=ysl[:, nsub * s : nsub * (s + 1)], in_=acc[:]
                        )
                if last:
                    eng = nc.sync if c0 % 2 == 0 else nc.scalar
                    eng.dma_start(out=y[:, ybase : ybase + NFREE], in_=ysl)
            if not last:
                nc.gpsimd.dma_start(
                    out=y[:, ci * DS * DS : (ci + 1) * DS * DS], in_=ysb[:]
                )

    nc.compile()
    return nc


def _get_compiled():
    if "nc" not in _CACHE:
        _CACHE["nc"] = build_nc(CI_PER_CORE)
    return _CACHE["nc"]


def _canonical_ijk(ijk):
    n = D * D * D
    if ijk.shape != (n, 3):
        return False
    r = np.arange(n, dtype=np.int64)
    return (
        np.array_equal(ijk[:, 0], (r // (D * D)).astype(ijk.dtype))
        and np.array_equal(ijk[:, 1], ((r // D) % D).astype(ijk.dtype))
        and np.array_equal(ijk[:, 2], (r % D).astype(ijk.dtype))
    )


def _prepare_x(in_data, ijk):
    """Return x in canonical dense-grid row order.

    For the expected (canonical arange) ijk this is in_data itself. For any
    other ijk, pre-permute on host so row r holds the fine voxel that the
    canonical layout would put there.
    """
    ijk = np.asarray(ijk)
    if _canonical_ijk(ijk):
        return in_data
    ijk64 = ijk.astype(np.int64)
    down = ijk64 // 2
    local = ijk64 - down * 2
    flat = (
        (down[:, 0] * DS * DS + down[:, 1] * DS + down[:, 2]) * 8
        + local[:, 0] * 4
        + local[:, 1] * 2
        + local[:, 2]
    )
    n = D * D * D
    pos = np.empty(n, dtype=np.int64)
    pos[flat] = np.arange(n, dtype=np.int64)
    r = np.arange(n, dtype=np.int64)
    i, j, k = r // (D * D), (r // D) % D, r % D
    f_canon = (
        ((i // 2) * DS * DS + (j // 2) * DS + (k // 2)) * 8
        + (i % 2) * 4
        + (j % 2) * 2
        + (k % 2)
    )
    return in_data[pos[f_canon]]


def _pack_x(x):
    """[N, 64] fp32 (canonical order) -> [8, 36, 128, 1536] bf16 chunk blocks.

    Core d, chunk ci*6 + c0, partition 64*lk + c, column li*768 + jj*48 + kc
    holds bf16(x[(12d + 2*ci + li, 16*c0 + jj, 2*kc + lk), c]).
    """
    import ml_dtypes

    xb = np.asarray(x, dtype=np.float32).astype(ml_dtypes.bfloat16)
    # axes: [d, ci, li, c0, jj, kc, lk, c]
    x8 = xb.reshape(N_CORES, CI_PER_CORE, 2, NCHUNKS_CJ, 2 * CJ_BLK, D // 2, 2, C)
    # -> [d, ci, c0, lk, c, li, jj, kc]
    return np.ascontiguousarray(x8.transpose(0, 1, 3, 6, 7, 2, 4, 5)).reshape(
        N_CORES, N_CHUNKS, 128, CHUNK_COLS
    )


def _pack_w(w_out):
    """[512, 64] -> [128, 4, 64] bf16: wt[64*lk + c, 2*li + lj] = w_out[64*s + c]
    with s = 4*li + 2*lj + lk."""
    import ml_dtypes

    w5 = np.asarray(w_out, dtype=np.float32).reshape(2, 2, 2, C, C)
    return np.ascontiguousarray(w5.transpose(2, 3, 0, 1, 4)).reshape(
        128, 4, C
    ).astype(ml_dtypes.bfloat16)


def run_sharded(x, w_int, trace=False):
    from concourse.bass_utils import run_bass_kernel_spmd

    nc = _get_compiled()
    in_maps = [{"x": x[d], "w": w_int} for d in range(N_CORES)]
    res = run_bass_kernel_spmd(
        nc, in_maps, list(range(N_CORES)), trace=trace
    )
    out = np.concatenate(
        [np.asarray(res.results[d]["y"]).T.astype(np.float32) for d in range(N_CORES)],
        axis=0,
    )
    return out, res


def prepare_inputs(in_data, ijk, w_out):
    in_data = np.asarray(in_data, dtype=np.float32)
    x = _pack_x(_prepare_x(in_data, ijk))
    return x, _pack_w(w_out)


def kernel(in_data, ijk, w_out):
    x, w_int = prepare_inputs(in_data, ijk, w_out)
    out, _ = run_sharded(x, w_int, trace=False)
    return out


# revision 24
# speedup vs baseline: 1.0177x; 1.0177x over previous
"""Trainium2 Bass kernel for DownSamplingSpatial2Channel (space-to-depth + projection).

Computes, for a dense 96^3 voxel grid with 64 channels:
    out[d] = sum_s in_data[r(d, s)] @ W_s
where d indexes the 48^3 coarse grid, s the 8 sub-voxels of a 2x2x2 block,
r(d, s) the fine-grid row, and W_s = w_out[64*s : 64*s+64, :].

Sharding: data-parallel over fine-grid i-planes. Core d owns fine planes
[12d, 12d+12) and coarse planes [6d, 6d+6).

Host prep (not on the device critical path): in_data is rounded to bf16 and
laid out channel-major with the k-subvoxel pair folded onto the partition
axis, blocked so each (coarse plane ci, j-chunk c0) is one contiguous
[128, 1536] DRAM block:
  x[d][ci*6 + c0][64*lk + c, li*768 + jj*48 + kc]
      = in_data[(12d + 2*ci + li, 16*c0 + jj, 2*kc + lk), c].
The [512,64] projection is regrouped as 4 stationaries W'_{li,lj} [128,64]
whose contraction dim spans (lk, c) — each matmul contracts a fully dense
K=128 (two sub-voxels at once) and the space-to-depth scatter is entirely
strided access patterns on the moving operand.

Device pipeline per core (36 chunks, uniform):
  1. One plain 384 KB DMA load per chunk ([128, 1536] bf16, alternating the
     two HWDGE rings) — contiguous, full HBM bandwidth, no XBAR. Chunk
     granularity keeps the PE no more than ~1 chunk behind the load stream,
     so the post-last-byte tail is one chunk's matmul + cast + store.
  2. Per chunk: 4 accumulating bf16 matmuls (K=128, N=384) into PSUM
     Y.T [64 out-ch, 384 voxels], then a DVE fp32->bf16 cast into SBUF.
  3. One [64, 2304] store per coarse plane on the SWDGE ring; the last
     coarse plane stores per chunk on the (by then idle) HWDGE rings.
     Output stays transposed ([64, Nd] bf16); the host transposes back and
     upcasts when unsharding.
"""

import numpy as np

D = 96            # fine grid edge
DS = 48           # coarse grid edge
C = 64            # channels
N_CORES = 8
PLANES_PER_CORE = D // N_CORES          # 12 fine i-planes
CI_PER_CORE = DS // N_CORES             # 6 coarse i-planes
ND = CI_PER_CORE * DS * DS              # 13824 coarse rows per core
CJ_BLK = 8                              # coarse j-lines per matmul chunk
NFREE = CJ_BLK * DS                     # 384 moving free dim
NCHUNKS_CJ = DS // CJ_BLK               # 6 chunks per coarse plane
CHUNK_COLS = 2 * 2 * CJ_BLK * (D // 2)  # 1536 packed cols per chunk
N_CHUNKS = CI_PER_CORE * NCHUNKS_CJ     # 36 chunks per core

_CACHE = {}


def build_nc(n_ci=CI_PER_CORE):
    from contextlib import ExitStack

    import concourse.bass as bass  # noqa: F401
    import concourse.mybir as mybir
    import concourse.tile as tile
    from concourse import bacc

    dt = mybir.dt
    f32, bf16 = dt.float32, dt.bfloat16
    n_chunks = n_ci * NCHUNKS_CJ
    nd = n_ci * DS * DS

    nc = bacc.Bacc(
        "TRN2",
        target_bir_lowering=False,
        debug=False,
        num_devices=N_CORES,
    )
    x = nc.dram_tensor("x", [n_chunks, 128, CHUNK_COLS], bf16, kind="ExternalInput").ap()
    # w slots t = 2*li + lj: stationary W'_t with K spanning (lk, c)
    w = nc.dram_tensor("w", [128, 4, C], bf16, kind="ExternalInput").ap()
    y = nc.dram_tensor("y", [C, nd], bf16, kind="ExternalOutput").ap()

    with tile.TileContext(nc) as tc, ExitStack() as ctx:
        const = ctx.enter_context(tc.tile_pool(name="const", bufs=1))
        xtpool = ctx.enter_context(tc.tile_pool(name="xt", bufs=n_chunks))
        ypool = ctx.enter_context(tc.tile_pool(name="ysb", bufs=2))
        apsum = ctx.enter_context(tc.tile_pool(name="acc", bufs=4, space="PSUM"))

        wt = const.tile([128, 4, C], bf16, tag="wt")
        nc.gpsimd.dma_start(out=wt[:], in_=w)

        # PE warm-up: dummy matmuls on a zeroed scratch tile keep the PE
        # busy through the HAM SHORT window during the load ramp, so the
        # first real matmuls run at 2.4 GHz instead of the cold 1.2 GHz.
        # The scratch PSUM result is never read.
        warm = const.tile([128, NFREE], bf16, tag="warm")
        nc.vector.memset(warm[:], 0)
        wacc = apsum.tile([C, NFREE], f32, tag="wacc")
        for _ in range(9):
            nc.tensor.matmul(
                wacc[:], warm[:, 0:C], warm[:], start=True, stop=True
            )

        xt_tiles = {}
        for k in range(n_chunks):
            xt = xtpool.tile([128, CHUNK_COLS], bf16, tag="xt")
            xt_tiles[k] = xt[:]
            eng = nc.sync if k % 2 == 0 else nc.scalar
            if k < n_chunks - 2:
                eng.dma_start(out=xt[:], in_=x[k])
            else:
                # Last two chunks: four quarter-loads each, so the DMA
                # completion receipt and most of the chunk's matmuls overlap
                # earlier quarters instead of sitting on the end-of-kernel
                # dependency chain.
                xk = x[k].rearrange("p (li j kc) -> p li j kc", li=2, j=2 * CJ_BLK)
                xtv = xt[:].rearrange("p (li j kc) -> p li j kc", li=2, j=2 * CJ_BLK)
                for s in range(4):
                    eng.dma_start(
                        out=xtv[:, :, 4 * s : 4 * (s + 1), :],
                        in_=xk[:, :, 4 * s : 4 * (s + 1), :],
                    )

        for ci in range(n_ci):
            last = ci == n_ci - 1
            ysb = ypool.tile([C, NCHUNKS_CJ * NFREE], bf16, tag="ysb")
            for c0 in range(NCHUNKS_CJ):
                k = ci * NCHUNKS_CJ + c0
                xt4 = xt_tiles[k].rearrange(
                    "p (li j kc) -> p li j kc", li=2, j=2 * CJ_BLK
                )
                ysl = ysb[:, NFREE * c0 : NFREE * (c0 + 1)]
                ybase = ci * DS * DS + NFREE * c0
                if k < n_chunks - 2:
                    acc = apsum.tile([C, NFREE], f32, tag="acc")
                    for li in range(2):
                        for lj in range(2):
                            t = 2 * li + lj
                            nc.tensor.matmul(
                                acc[:],
                                wt[:, t, :],
                                xt4[:, li, lj : 2 * CJ_BLK : 2, :],
                                start=(t == 0),
                                stop=(t == 3),
                            )
                    nc.vector.tensor_copy(out=ysl, in_=acc[:])
                else:
                    # Quarter-granular compute matching the quarter-loads.
                    nsub = NFREE // 4
                    for s in range(4):
                        acc = apsum.tile([C, nsub], f32, tag="acc")
                        for li in range(2):
                            for lj in range(2):
                                t = 2 * li + lj
                                nc.tensor.matmul(
                                    acc[:],
                                    wt[:, t, :],
                                    xt4[:, li, 4 * s + lj : 4 * s + 4 : 2, :],
                                    start=(t == 0),
                                    stop=(t == 3),
                                )
                        nc.vector.tensor_copy(
                            out=ysl[:, nsub * s : nsub * (s + 1)], in_=acc[:]
                        )
                if last:
                    eng = nc.sync if c0 % 2 == 0 else nc.scalar
                    eng.dma_start(out=y[:, ybase : ybase + NFREE], in_=ysl)
            if not last:
                nc.gpsimd.dma_start(
                    out=y[:, ci * DS * DS : (ci + 1) * DS * DS], in_=ysb[:]
                )

    nc.compile()
    return nc


def _get_compiled():
    if "nc" not in _CACHE:
        _CACHE["nc"] = build_nc(CI_PER_CORE)
    return _CACHE["nc"]


def _canonical_ijk(ijk):
    n = D * D * D
    if ijk.shape != (n, 3):
        return False
    r = np.arange(n, dtype=np.int64)
    return (
        np.array_equal(ijk[:, 0], (r // (D * D)).astype(ijk.dtype))
        and np.array_equal(ijk[:, 1], ((r // D) % D).astype(ijk.dtype))
        and np.array_equal(ijk[:, 2], (r % D).astype(ijk.dtype))
    )


def _prepare_x(in_data, ijk):
    """Return x in canonical dense-grid row order.

    For the expected (canonical arange) ijk this is in_data itself. For any
    other ijk, pre-permute on host so row r holds the fine voxel that the
    canonical layout would put there.
    """
    ijk = np.asarray(ijk)
    if _canonical_ijk(ijk):
        return in_data
    ijk64 = ijk.astype(np.int64)
    down = ijk64 // 2
    local = ijk64 - down * 2
    flat = (
        (down[:, 0] * DS * DS + down[:, 1] * DS + down[:, 2]) * 8
        + local[:, 0] * 4
        + local[:, 1] * 2
        + local[:, 2]
    )
    n = D * D * D
    pos = np.empty(n, dtype=np.int64)
    pos[flat] = np.arange(n, dtype=np.int64)
    r = np.arange(n, dtype=np.int64)
    i, j, k = r // (D * D), (r // D) % D, r % D
    f_canon = (
        ((i // 2) * DS * DS + (j // 2) * DS + (k // 2)) * 8
        + (i % 2) * 4
        + (j % 2) * 2
        + (k % 2)
    )
    return in_data[pos[f_canon]]


def _pack_x(x):
    """[N, 64] fp32 (canonical order) -> [8, 36, 128, 1536] bf16 chunk blocks.

    Core d, chunk ci*6 + c0, partition 64*lk + c, column li*768 + jj*48 + kc
    holds bf16(x[(12d + 2*ci + li, 16*c0 + jj, 2*kc + lk), c]).
    """
    import ml_dtypes

    xb = np.asarray(x, dtype=np.float32).astype(ml_dtypes.bfloat16)
    # axes: [d, ci, li, c0, jj, kc, lk, c]
    x8 = xb.reshape(N_CORES, CI_PER_CORE, 2, NCHUNKS_CJ, 2 * CJ_BLK, D // 2, 2, C)
    # -> [d, ci, c0, lk, c, li, jj, kc]
    return np.ascontiguousarray(x8.transpose(0, 1, 3, 6, 7, 2, 4, 5)).reshape(
        N_CORES, N_CHUNKS, 128, CHUNK_COLS
    )


def _pack_w(w_out):
    """[512, 64] -> [128, 4, 64] bf16: wt[64*lk + c, 2*li + lj] = w_out[64*s + c]
    with s = 4*li + 2*lj + lk."""
    import ml_dtypes

    w5 = np.asarray(w_out, dtype=np.float32).reshape(2, 2, 2, C, C)
    return np.ascontiguousarray(w5.transpose(2, 3, 0, 1, 4)).reshape(
        128, 4, C
    ).astype(ml_dtypes.bfloat16)


def run_sharded(x, w_int, trace=False):
    from concourse.bass_utils import run_bass_kernel_spmd

    nc = _get_compiled()
    in_maps = [{"x": x[d], "w": w_int} for d in range(N_CORES)]
    res = run_bass_kernel_spmd(
        nc, in_maps, list(range(N_CORES)), trace=trace
    )
    out = np.concatenate(
        [np.asarray(res.results[d]["y"]).T.astype(np.float32) for d in range(N_CORES)],
        axis=0,
    )
    return out, res


def prepare_inputs(in_data, ijk, w_out):
    in_data = np.asarray(in_data, dtype=np.float32)
    x = _pack_x(_prepare_x(in_data, ijk))
    return x, _pack_w(w_out)


def kernel(in_data, ijk, w_out):
    x, w_int = prepare_inputs(in_data, ijk, w_out)
    out, _ = run_sharded(x, w_int, trace=False)
    return out
